# revision 3
# baseline (speedup 1.0000x reference)
"""Trainium2 Bass kernel for nn_Autocorrelation (B=16, L=1024, D=512, H=8, dh=64).

Self-contained: kernel(**inputs) -> np.ndarray [16, 1024, 512].

Algorithm notes:
- The reference broadcasts ONE projection across all 8 heads, so the real work
  is per (batch, dh) row: 16*64 = 1024 rows of length L=1024. Each core takes
  2 batches = 128 rows (exactly the SBUF partition count).
- FFT correlation is replaced by dense DFT matmuls against symmetric cos/sin
  matrices C/S (C[a,b]=cos(2*pi*a*b/L), S[a,b]=-sin(...)); the same matrices
  serve forward and inverse transforms.
- ifft's 1/L is folded into the q projection (Wq/L, bq/L) for stage 1 and into
  the softmax weights (w/L) for stage 2, so corr keeps reference scale.
- top-13 via DVE max/max_index/match_replace (top-8 per instruction).
- The lag-gather + weighted sum is a second circular correlation with the
  sparse weight vector s (softmax weights scattered at the top-k lags).
"""

import threading

import numpy as np

L = 1024
D = 512
DH = 64
BLOC = 2          # batches per core
B = 16
H = 8
KTOP = 13
NCORES = 8

F32MAX_NEG = -1.0e30


def _build_nc(cfg=None):
    import concourse.bass as bass
    import concourse.mybir as mybir
    import concourse.tile as tile
    from concourse import bacc
    from concourse.masks import make_identity

    f32 = mybir.dt.float32
    f32r = mybir.dt.float32r
    u32 = mybir.dt.uint32
    i32 = mybir.dt.int32

    nc = bacc.Bacc("TRN2", target_bir_lowering=False, debug=False, num_devices=NCORES)

    Qd = nc.declare_dram_parameter("Q", [BLOC, L, D], f32, isOutput=False)
    Kd = nc.declare_dram_parameter("K", [BLOC, L, D], f32, isOutput=False)
    Vd = nc.declare_dram_parameter("V", [BLOC, L, D], f32, isOutput=False)
    Wql = nc.declare_dram_parameter("Wql", [D, DH], f32, isOutput=False)  # Wq/L
    bql = nc.declare_dram_parameter("bql", [DH], f32, isOutput=False)      # bq/L
    Wqu = nc.declare_dram_parameter("Wqu", [D, DH], f32, isOutput=False)
    bqu = nc.declare_dram_parameter("bqu", [DH], f32, isOutput=False)
    Cm = nc.declare_dram_parameter("Cm", [L, L], f32, isOutput=False)
    Sm = nc.declare_dram_parameter("Sm", [L, L], f32, isOutput=False)
    outd = nc.declare_dram_parameter("out", [BLOC, L, D], f32, isOutput=True)

    def bcast_part(ap, n):
        # replicate a [free...] DRAM AP across n partitions (step-0 partition dim)
        return bass.AP(tensor=ap.tensor, offset=ap.offset, ap=[[0, n]] + list(ap.ap))

    USE_F32R = bool(cfg and cfg.get("f32r"))

    def r(ap):
        return ap.bitcast(f32r) if USE_F32R else ap

    from contextlib import ExitStack

    with tile.TileContext(nc) as tc, ExitStack() as ctx:
        consts = ctx.enter_context(tc.tile_pool(name="consts", bufs=1))
        rowsp = ctx.enter_context(tc.tile_pool(name="rowsp", bufs=1))
        xpool = ctx.enter_context(tc.tile_pool(name="xpool", bufs=4))
        xtpool = ctx.enter_context(tc.tile_pool(name="xtpool", bufs=8))
        spec = ctx.enter_context(tc.tile_pool(name="spec", bufs=6))
        rowbig = ctx.enter_context(tc.tile_pool(name="rowbig", bufs=4))
        tmpbig = ctx.enter_context(tc.tile_pool(name="tmpbig", bufs=3))
        chunksp = ctx.enter_context(tc.tile_pool(name="chunksp", bufs=3))
        small = ctx.enter_context(tc.tile_pool(name="small", bufs=2))
        repp = ctx.enter_context(tc.tile_pool(name="repp", bufs=3))
        psum_big = ctx.enter_context(
            tc.tile_pool(name="psum_big", bufs=4, space="PSUM")
        )
        psum_t = ctx.enter_context(tc.tile_pool(name="psum_t", bufs=2, space="PSUM"))
        psum_pj = ctx.enter_context(tc.tile_pool(name="psum_pj", bufs=2, space="PSUM"))

        # ---- constants ----
        ident = consts.tile([128, 128], f32)
        make_identity(nc, ident)
        Csb = consts.tile([128, 8, L], f32)
        nc.sync.dma_start(out=Csb, in_=Cm[:, :].rearrange("(a p) x -> p a x", p=128))
        Ssb = consts.tile([128, 8, L], f32)
        nc.sync.dma_start(out=Ssb, in_=Sm[:, :].rearrange("(a p) x -> p a x", p=128))
        Wql_sb = consts.tile([128, 4, DH], f32)
        nc.sync.dma_start(out=Wql_sb, in_=Wql[:, :].rearrange("(c p) h -> p c h", p=128))
        Wqu_sb = consts.tile([128, 4, DH], f32)
        nc.sync.dma_start(out=Wqu_sb, in_=Wqu[:, :].rearrange("(c p) h -> p c h", p=128))
        bql_bc = consts.tile([128, DH], f32)
        nc.sync.dma_start(out=bql_bc, in_=bcast_part(bql[:], 128))
        bqu_bc = consts.tile([128, DH], f32)
        nc.sync.dma_start(out=bqu_bc, in_=bcast_part(bqu[:], 128))
        iota_f = consts.tile([128, L], f32)
        iota_i = small.tile([128, L], i32, tag="iota_i", bufs=1)
        nc.gpsimd.iota(iota_i, pattern=[[1, L]], base=0, channel_multiplier=0)
        nc.vector.tensor_copy(iota_f, iota_i)

        # ---- projections: rows_t[p, jc, 64*b+dh] = (X_b @ W + bias)[128*jc+p, dh]
        rows_q = rowsp.tile([128, 8, 128], f32)
        rows_k = rowsp.tile([128, 8, 128], f32)
        rows_v = rowsp.tile([128, 8, 128], f32)
        for dram, Wsb, bbc, dst in (
            (Qd, Wql_sb, bql_bc, rows_q),
            (Kd, Wqu_sb, bqu_bc, rows_k),
            (Vd, Wqu_sb, bqu_bc, rows_v),
        ):
            for b in range(BLOC):
                for lt in range(8):
                    x_nat = xpool.tile([128, D], f32, tag="x_nat")
                    nc.sync.dma_start(
                        out=x_nat, in_=dram[b, lt * 128 : (lt + 1) * 128, :]
                    )
                    ps = psum_pj.tile([128, DH], f32, tag="pj")
                    for dc in range(4):
                        tp = psum_t.tile([128, 128], f32, tag="tr")
                        nc.tensor.transpose(
                            tp, x_nat[:, dc * 128 : (dc + 1) * 128], ident
                        )
                        xT = xtpool.tile([128, 128], f32, tag="xT")
                        nc.any.tensor_copy(xT, tp)
                        nc.tensor.matmul(
                            ps,
                            lhsT=xT,
                            rhs=Wsb[:, dc, :],
                            start=(dc == 0),
                            stop=(dc == 3),
                        )
                    nc.vector.tensor_add(dst[:, lt, 64 * b : 64 * b + 64], ps, bbc)

        # ---- forward DFT from row-chunk layout [128(j), jc, 128(r)] ----
        def fwd(rows_src, nm):
            psr0 = psum_big.tile([128, 512], f32, tag="big")
            psr1 = psum_big.tile([128, 512], f32, tag="big")
            psi0 = psum_big.tile([128, 512], f32, tag="big")
            psi1 = psum_big.tile([128, 512], f32, tag="big")
            for jc in range(8):
                lhsT = r(rows_src[:, jc, :])
                st, sp = jc == 0, jc == 7
                nc.tensor.matmul(psr0, lhsT=lhsT, rhs=r(Csb[:, jc, 0:512]), start=st, stop=sp)
                nc.tensor.matmul(psr1, lhsT=lhsT, rhs=r(Csb[:, jc, 512:1024]), start=st, stop=sp)
                nc.tensor.matmul(psi0, lhsT=lhsT, rhs=r(Ssb[:, jc, 0:512]), start=st, stop=sp)
                nc.tensor.matmul(psi1, lhsT=lhsT, rhs=r(Ssb[:, jc, 512:1024]), start=st, stop=sp)
            re_ = spec.tile([128, L], f32, tag="spec", name=f"{nm}_re")
            im_ = spec.tile([128, L], f32, tag="spec", name=f"{nm}_im")
            nc.any.tensor_copy(re_[:, 0:512], psr0)
            nc.any.tensor_copy(re_[:, 512:1024], psr1)
            nc.any.tensor_copy(im_[:, 0:512], psi0)
            nc.any.tensor_copy(im_[:, 512:1024], psi1)
            return re_, im_

        # ---- transpose [128, 1024] row tensor into chunk layout [128, fc, 128] ----
        def to_chunks(src, nm):
            dstT = chunksp.tile([128, 8, 128], f32, tag="chT", name=f"{nm}_T")
            for fc in range(8):
                tp = psum_t.tile([128, 128], f32, tag="tr")
                nc.tensor.transpose(tp, src[:, fc * 128 : (fc + 1) * 128], ident)
                nc.any.tensor_copy(dstT[:, fc, :], tp)
            return dstT

        # ---- inverse DFT: sum_f ReT[f,r]*C[f,tau] + ImT[f,r]*S[f,tau] ----
        def inv(ReT, ImT, nm):
            ps0 = psum_big.tile([128, 512], f32, tag="big")
            ps1 = psum_big.tile([128, 512], f32, tag="big")
            for fc in range(8):
                st, sp = fc == 0, fc == 7
                nc.tensor.matmul(ps0, lhsT=r(ReT[:, fc, :]), rhs=r(Csb[:, fc, 0:512]), start=st, stop=False)
                nc.tensor.matmul(ps0, lhsT=r(ImT[:, fc, :]), rhs=r(Ssb[:, fc, 0:512]), start=False, stop=sp)
                nc.tensor.matmul(ps1, lhsT=r(ReT[:, fc, :]), rhs=r(Csb[:, fc, 512:1024]), start=st, stop=False)
                nc.tensor.matmul(ps1, lhsT=r(ImT[:, fc, :]), rhs=r(Ssb[:, fc, 512:1024]), start=False, stop=sp)
            res = rowbig.tile([128, L], f32, tag="row", name=f"{nm}_res")
            nc.any.tensor_copy(res[:, 0:512], ps0)
            nc.any.tensor_copy(res[:, 512:1024], ps1)
            return res

        Qr, Qi = fwd(rows_q, "q")
        Kr, Ki = fwd(rows_k, "k")

        # pointwise: X = Qhat * conj(Khat)
        t1 = tmpbig.tile([128, L], f32, tag="tmp")
        t2 = tmpbig.tile([128, L], f32, tag="tmp")
        nc.vector.tensor_mul(t1, Qr, Kr)
        nc.vector.tensor_mul(t2, Qi, Ki)
        XR = rowbig.tile([128, L], f32, tag="row")
        nc.vector.tensor_add(XR, t1, t2)
        t3 = tmpbig.tile([128, L], f32, tag="tmp")
        t4 = tmpbig.tile([128, L], f32, tag="tmp")
        nc.vector.tensor_mul(t3, Qi, Kr)
        nc.vector.tensor_mul(t4, Qr, Ki)
        XI = rowbig.tile([128, L], f32, tag="row")
        nc.vector.tensor_sub(XI, t3, t4)

        XRT = to_chunks(XR, "xr")
        XIT = to_chunks(XI, "xi")
        corr = inv(XRT, XIT, "corr")

        # forward DFT of v early (overlaps top-k on the PE side)
        Vr, Vi = fwd(rows_v, "v")

        # ---- top-13 + softmax ----
        vals16 = small.tile([128, 16], f32, tag="vals")
        idx16 = small.tile([128, 16], u32, tag="idx")
        nc.vector.max(out=vals16[:, 0:8], in_=corr)
        nc.vector.max_index(idx16[:, 0:8], vals16[:, 0:8], corr)
        nc.vector.match_replace(
            out=corr, in_to_replace=vals16[:, 0:8], in_values=corr,
            imm_value=F32MAX_NEG,
        )
        nc.vector.max(out=vals16[:, 8:16], in_=corr)
        nc.vector.max_index(idx16[:, 8:16], vals16[:, 8:16], corr)

        import concourse.mybir as mybir2

        negm = small.tile([128, 1], f32, tag="negm")
        nc.vector.tensor_scalar_mul(negm, vals16[:, 0:1], -1.0)
        e13 = small.tile([128, KTOP], f32, tag="e13")
        ssum = small.tile([128, 1], f32, tag="ssum")
        nc.scalar.activation(
            e13, vals16[:, 0:KTOP], mybir2.ActivationFunctionType.Exp,
            bias=negm, scale=1.0, accum_out=ssum,
        )
        rs = small.tile([128, 1], f32, tag="rs")
        nc.vector.reciprocal(rs, ssum)
        w13 = small.tile([128, KTOP], f32, tag="w13")
        nc.vector.tensor_scalar(
            w13, e13, scalar1=rs, scalar2=1.0 / L,
            op0=mybir2.AluOpType.mult, op1=mybir2.AluOpType.mult,
        )
        idxf = small.tile([128, 16], f32, tag="idxf")
        nc.vector.tensor_copy(idxf, idx16)

        # ---- scatter s[r, m] = sum_k w_k * (m == idx_k) ----
        s_t = rowbig.tile([128, L], f32, tag="row")
        nc.vector.memset(s_t, 0.0)
        for k in range(KTOP):
            tk = tmpbig.tile([128, L], f32, tag="tmp")
            nc.vector.tensor_scalar(
                tk, iota_f, scalar1=idxf[:, k : k + 1], scalar2=w13[:, k : k + 1],
                op0=mybir2.AluOpType.is_equal, op1=mybir2.AluOpType.mult,
            )
            nc.vector.tensor_add(s_t, s_t, tk)

        sT = to_chunks(s_t, "s")
        Sr, Si = fwd(sT, "sp")

        # pointwise: Y = Vhat * conj(Shat)
        u1 = tmpbig.tile([128, L], f32, tag="tmp")
        u2 = tmpbig.tile([128, L], f32, tag="tmp")
        nc.vector.tensor_mul(u1, Vr, Sr)
        nc.vector.tensor_mul(u2, Vi, Si)
        YR = rowbig.tile([128, L], f32, tag="row")
        nc.vector.tensor_add(YR, u1, u2)
        u3 = tmpbig.tile([128, L], f32, tag="tmp")
        u4 = tmpbig.tile([128, L], f32, tag="tmp")
        nc.vector.tensor_mul(u3, Vi, Sr)
        nc.vector.tensor_mul(u4, Vr, Si)
        YI = rowbig.tile([128, L], f32, tag="row")
        nc.vector.tensor_sub(YI, u3, u4)

        YRT = to_chunks(YR, "yr")
        YIT = to_chunks(YI, "yi")
        agg = inv(YRT, YIT, "agg")

        # ---- transpose agg -> [tau, r], replicate 8 heads, DMA out ----
        for tt in range(8):
            tp = psum_t.tile([128, 128], f32, tag="tr")
            nc.tensor.transpose(tp, agg[:, tt * 128 : (tt + 1) * 128], ident)
            at = xtpool.tile([128, 128], f32, tag="at")
            nc.any.tensor_copy(at, tp)
            for b in range(BLOC):
                rep = repp.tile([128, H, DH], f32, tag="rep")
                col = at[:, 64 * b : 64 * b + 64]
                src = bass.AP(
                    tensor=col.tensor, offset=col.offset,
                    ap=[list(col.ap[0]), [0, H], list(col.ap[1])],
                )
                nc.vector.tensor_copy(rep, src)
                nc.sync.dma_start(
                    out=outd[b, tt * 128 : (tt + 1) * 128, :].rearrange(
                        "t (h d) -> t h d", h=H
                    ),
                    in_=rep,
                )

    nc.compile()
    return nc


_cache = threading.Lock(), {}


def _get_nc():
    lock, store = _cache
    with lock:
        if "nc" not in store:
            store["nc"] = _build_nc()
        return store["nc"]


def _make_consts():
    j = np.arange(L, dtype=np.float64)
    ang = 2.0 * np.pi * np.outer(j, j) / L
    Cmat = np.cos(ang).astype(np.float32)
    Smat = (-np.sin(ang)).astype(np.float32)
    return Cmat, Smat


def kernel(Q, K, V, Wq, bq):
    from concourse.bass_utils import run_bass_kernel_spmd

    Q = np.ascontiguousarray(Q, np.float32)
    K = np.ascontiguousarray(K, np.float32)
    V = np.ascontiguousarray(V, np.float32)
    Wq = np.ascontiguousarray(Wq, np.float32)
    bq = np.ascontiguousarray(bq, np.float32)

    nc = _get_nc()
    Cmat, Smat = _make_consts()
    Wql = (Wq / L).astype(np.float32)
    bql = (bq / L).astype(np.float32)

    in_maps = []
    for c in range(NCORES):
        sl = slice(BLOC * c, BLOC * (c + 1))
        in_maps.append(
            {
                "Q": Q[sl], "K": K[sl], "V": V[sl],
                "Wql": Wql, "bql": bql, "Wqu": Wq, "bqu": bq,
                "Cm": Cmat, "Sm": Smat,
            }
        )
    res = run_bass_kernel_spmd(nc, in_maps, list(range(NCORES)))
    return np.concatenate([res.results[i]["out"] for i in range(NCORES)], axis=0)


# revision 6
# speedup vs baseline: 1.1423x; 1.1423x over previous
"""Trainium2 Bass kernel for nn_Autocorrelation (B=16, L=1024, D=512, H=8, dh=64).

Self-contained: kernel(**inputs) -> np.ndarray [16, 1024, 512].

Algorithm notes:
- The reference broadcasts ONE projection across all 8 heads, so the real work
  is per (batch, dh) row: 16*64 = 1024 rows of length L=1024. Each core takes
  2 batches = 128 rows (exactly the SBUF partition count).
- FFT correlation is replaced by dense DFT matmuls against symmetric cos/sin
  matrices C/S (C[a,b]=cos(2*pi*a*b/L), S[a,b]=-sin(...)); the same matrices
  serve forward and inverse transforms.
- ifft's 1/L is folded into the q projection (Wq/L, bq/L) for stage 1 and into
  the softmax weights (w/L) for stage 2, so corr keeps reference scale.
- top-13 via DVE max/max_index/match_replace (top-8 per instruction).
- The lag-gather + weighted sum is a second circular correlation with the
  sparse weight vector s (softmax weights scattered at the top-k lags).
"""

import threading

import numpy as np

L = 1024
D = 512
DH = 64
BLOC = 2          # batches per core
B = 16
H = 8
KTOP = 13
NCORES = 8

F32MAX_NEG = -1.0e30


def _build_nc(cfg=None):
    import concourse.bass as bass
    import concourse.mybir as mybir
    import concourse.tile as tile
    from concourse import bacc
    from concourse.masks import make_identity

    f32 = mybir.dt.float32
    f32r = mybir.dt.float32r
    u32 = mybir.dt.uint32
    i32 = mybir.dt.int32

    cfg = cfg or {}
    USE_F32R = cfg.get("f32r", True)
    mm_dt = f32r if USE_F32R else f32

    nc = bacc.Bacc("TRN2", target_bir_lowering=False, debug=False, num_devices=NCORES)

    Qd = nc.declare_dram_parameter("Q", [BLOC, L, D], f32, isOutput=False)
    Kd = nc.declare_dram_parameter("K", [BLOC, L, D], f32, isOutput=False)
    Vd = nc.declare_dram_parameter("V", [BLOC, L, D], f32, isOutput=False)
    Wql = nc.declare_dram_parameter("Wql", [D, DH], f32, isOutput=False)  # Wq/L
    bql = nc.declare_dram_parameter("bql", [DH], f32, isOutput=False)      # bq/L
    Wqu = nc.declare_dram_parameter("Wqu", [D, DH], f32, isOutput=False)
    bqu = nc.declare_dram_parameter("bqu", [DH], f32, isOutput=False)
    Cm = nc.declare_dram_parameter("Cm", [L, L], mm_dt, isOutput=False)
    Sm = nc.declare_dram_parameter("Sm", [L, L], mm_dt, isOutput=False)
    outd = nc.declare_dram_parameter("out", [BLOC, L, D], f32, isOutput=True)

    def bcast_part(ap, n):
        # replicate a [free...] DRAM AP across n partitions (step-0 partition dim)
        return bass.AP(tensor=ap.tensor, offset=ap.offset, ap=[[0, n]] + list(ap.ap))

    def r(ap):
        return ap

    from contextlib import ExitStack

    with tile.TileContext(nc) as tc, ExitStack() as ctx:
        consts = ctx.enter_context(tc.tile_pool(name="consts", bufs=1))
        rowsp = ctx.enter_context(tc.tile_pool(name="rowsp", bufs=1))
        xpool = ctx.enter_context(tc.tile_pool(name="xpool", bufs=4))
        xtpool = ctx.enter_context(tc.tile_pool(name="xtpool", bufs=8))
        spec = ctx.enter_context(tc.tile_pool(name="spec", bufs=6))
        rowbig = ctx.enter_context(tc.tile_pool(name="rowbig", bufs=4))
        tmpbig = ctx.enter_context(tc.tile_pool(name="tmpbig", bufs=3))
        chunksp = ctx.enter_context(tc.tile_pool(name="chunksp", bufs=3))
        small = ctx.enter_context(tc.tile_pool(name="small", bufs=2))
        repp = ctx.enter_context(tc.tile_pool(name="repp", bufs=3))
        psum_big = ctx.enter_context(
            tc.tile_pool(name="psum_big", bufs=4, space="PSUM")
        )
        psum_t = ctx.enter_context(tc.tile_pool(name="psum_t", bufs=2, space="PSUM"))
        psum_pj = ctx.enter_context(tc.tile_pool(name="psum_pj", bufs=2, space="PSUM"))

        # ---- constants ----
        ident = consts.tile([128, 128], f32)
        make_identity(nc, ident)
        Csb = consts.tile([128, 8, L], mm_dt)
        nc.sync.dma_start(out=Csb, in_=Cm[:, :].rearrange("(a p) x -> p a x", p=128))
        Ssb = consts.tile([128, 8, L], mm_dt)
        nc.sync.dma_start(out=Ssb, in_=Sm[:, :].rearrange("(a p) x -> p a x", p=128))
        Wql_sb = consts.tile([128, 4, DH], f32)
        nc.sync.dma_start(out=Wql_sb, in_=Wql[:, :].rearrange("(c p) h -> p c h", p=128))
        Wqu_sb = consts.tile([128, 4, DH], f32)
        nc.sync.dma_start(out=Wqu_sb, in_=Wqu[:, :].rearrange("(c p) h -> p c h", p=128))
        bql_bc = consts.tile([128, DH], f32)
        nc.sync.dma_start(out=bql_bc, in_=bcast_part(bql[:], 128))
        bqu_bc = consts.tile([128, DH], f32)
        nc.sync.dma_start(out=bqu_bc, in_=bcast_part(bqu[:], 128))
        iota_f = consts.tile([128, L], f32)
        iota_i = small.tile([128, L], i32, tag="iota_i", bufs=1)
        nc.gpsimd.iota(iota_i, pattern=[[1, L]], base=0, channel_multiplier=0)
        nc.vector.tensor_copy(iota_f, iota_i)

        # ---- projections: rows_t[p, jc, 64*b+dh] = (X_b @ W + bias)[128*jc+p, dh]
        rows_q = rowsp.tile([128, 8, 128], mm_dt)
        rows_k = rowsp.tile([128, 8, 128], mm_dt)
        rows_v = rowsp.tile([128, 8, 128], mm_dt)
        for dram, Wsb, bbc, dst in (
            (Qd, Wql_sb, bql_bc, rows_q),
            (Kd, Wqu_sb, bqu_bc, rows_k),
            (Vd, Wqu_sb, bqu_bc, rows_v),
        ):
            for b in range(BLOC):
                for lt in range(8):
                    x_nat = xpool.tile([128, D], f32, tag="x_nat")
                    nc.sync.dma_start(
                        out=x_nat, in_=dram[b, lt * 128 : (lt + 1) * 128, :]
                    )
                    ps = psum_pj.tile([128, DH], f32, tag="pj")
                    for dc in range(4):
                        tp = psum_t.tile([128, 128], f32, tag="tr")
                        nc.tensor.transpose(
                            tp, x_nat[:, dc * 128 : (dc + 1) * 128], ident
                        )
                        xT = xtpool.tile([128, 128], f32, tag="xT")
                        nc.any.tensor_copy(xT, tp)
                        nc.tensor.matmul(
                            ps,
                            lhsT=xT,
                            rhs=Wsb[:, dc, :],
                            start=(dc == 0),
                            stop=(dc == 3),
                        )
                    nc.vector.tensor_add(dst[:, lt, 64 * b : 64 * b + 64], ps, bbc)

        # ---- forward DFT from row-chunk layout [128(j), jc, 128(r)] ----
        def fwd(rows_src, nm):
            psr0 = psum_big.tile([128, 512], f32, tag="big")
            psr1 = psum_big.tile([128, 512], f32, tag="big")
            psi0 = psum_big.tile([128, 512], f32, tag="big")
            psi1 = psum_big.tile([128, 512], f32, tag="big")
            for jc in range(8):
                lhsT = r(rows_src[:, jc, :])
                st, sp = jc == 0, jc == 7
                nc.tensor.matmul(psr0, lhsT=lhsT, rhs=r(Csb[:, jc, 0:512]), start=st, stop=sp)
                nc.tensor.matmul(psr1, lhsT=lhsT, rhs=r(Csb[:, jc, 512:1024]), start=st, stop=sp)
                nc.tensor.matmul(psi0, lhsT=lhsT, rhs=r(Ssb[:, jc, 0:512]), start=st, stop=sp)
                nc.tensor.matmul(psi1, lhsT=lhsT, rhs=r(Ssb[:, jc, 512:1024]), start=st, stop=sp)
            re_ = spec.tile([128, L], f32, tag="spec", name=f"{nm}_re")
            im_ = spec.tile([128, L], f32, tag="spec", name=f"{nm}_im")
            nc.any.tensor_copy(re_[:, 0:512], psr0)
            nc.any.tensor_copy(re_[:, 512:1024], psr1)
            nc.any.tensor_copy(im_[:, 0:512], psi0)
            nc.any.tensor_copy(im_[:, 512:1024], psi1)
            return re_, im_

        # ---- transpose [128, 1024] row tensor into chunk layout [128, fc, 128] ----
        def to_chunks(src, nm):
            dstT = chunksp.tile([128, 8, 128], mm_dt, tag="chT", name=f"{nm}_T")
            for fc in range(8):
                tp = psum_t.tile([128, 128], f32, tag="tr")
                nc.tensor.transpose(tp, src[:, fc * 128 : (fc + 1) * 128], ident)
                nc.any.tensor_copy(dstT[:, fc, :], tp)
            return dstT

        # ---- inverse DFT: sum_f ReT[f,r]*C[f,tau] + ImT[f,r]*S[f,tau] ----
        def inv(ReT, ImT, nm):
            ps0 = psum_big.tile([128, 512], f32, tag="big")
            ps1 = psum_big.tile([128, 512], f32, tag="big")
            for fc in range(8):
                st, sp = fc == 0, fc == 7
                nc.tensor.matmul(ps0, lhsT=r(ReT[:, fc, :]), rhs=r(Csb[:, fc, 0:512]), start=st, stop=False)
                nc.tensor.matmul(ps0, lhsT=r(ImT[:, fc, :]), rhs=r(Ssb[:, fc, 0:512]), start=False, stop=sp)
                nc.tensor.matmul(ps1, lhsT=r(ReT[:, fc, :]), rhs=r(Csb[:, fc, 512:1024]), start=st, stop=False)
                nc.tensor.matmul(ps1, lhsT=r(ImT[:, fc, :]), rhs=r(Ssb[:, fc, 512:1024]), start=False, stop=sp)
            res = rowbig.tile([128, L], f32, tag="row", name=f"{nm}_res")
            nc.any.tensor_copy(res[:, 0:512], ps0)
            nc.any.tensor_copy(res[:, 512:1024], ps1)
            return res

        Qr, Qi = fwd(rows_q, "q")
        Kr, Ki = fwd(rows_k, "k")

        # pointwise: X = Qhat * conj(Khat)
        t1 = tmpbig.tile([128, L], f32, tag="tmp")
        t2 = tmpbig.tile([128, L], f32, tag="tmp")
        nc.vector.tensor_mul(t1, Qr, Kr)
        nc.vector.tensor_mul(t2, Qi, Ki)
        XR = rowbig.tile([128, L], f32, tag="row")
        nc.vector.tensor_add(XR, t1, t2)
        t3 = tmpbig.tile([128, L], f32, tag="tmp")
        t4 = tmpbig.tile([128, L], f32, tag="tmp")
        nc.vector.tensor_mul(t3, Qi, Kr)
        nc.vector.tensor_mul(t4, Qr, Ki)
        XI = rowbig.tile([128, L], f32, tag="row")
        nc.vector.tensor_sub(XI, t3, t4)

        XRT = to_chunks(XR, "xr")
        XIT = to_chunks(XI, "xi")
        corr = inv(XRT, XIT, "corr")

        # forward DFT of v early (overlaps top-k on the PE side)
        Vr, Vi = fwd(rows_v, "v")

        # ---- top-13 + softmax ----
        vals16 = small.tile([128, 16], f32, tag="vals")
        idx16 = small.tile([128, 16], u32, tag="idx")
        nc.vector.max(out=vals16[:, 0:8], in_=corr)
        nc.vector.max_index(idx16[:, 0:8], vals16[:, 0:8], corr)
        nc.vector.match_replace(
            out=corr, in_to_replace=vals16[:, 0:8], in_values=corr,
            imm_value=F32MAX_NEG,
        )
        nc.vector.max(out=vals16[:, 8:16], in_=corr)
        nc.vector.max_index(idx16[:, 8:16], vals16[:, 8:16], corr)

        import concourse.mybir as mybir2

        negm = small.tile([128, 1], f32, tag="negm")
        nc.vector.tensor_scalar_mul(negm, vals16[:, 0:1], -1.0)
        e13 = small.tile([128, KTOP], f32, tag="e13")
        ssum = small.tile([128, 1], f32, tag="ssum")
        nc.scalar.activation(
            e13, vals16[:, 0:KTOP], mybir2.ActivationFunctionType.Exp,
            bias=negm, scale=1.0, accum_out=ssum,
        )
        rs = small.tile([128, 1], f32, tag="rs")
        nc.vector.reciprocal(rs, ssum)
        w13 = small.tile([128, KTOP], f32, tag="w13")
        nc.vector.tensor_scalar(
            w13, e13, scalar1=rs, scalar2=1.0 / L,
            op0=mybir2.AluOpType.mult, op1=mybir2.AluOpType.mult,
        )
        idxf = small.tile([128, 16], f32, tag="idxf")
        nc.vector.tensor_copy(idxf, idx16)

        # ---- scatter s[r, m] = sum_k w_k * (m == idx_k) ----
        s_t = rowbig.tile([128, L], f32, tag="row")
        nc.vector.memset(s_t, 0.0)
        for k in range(KTOP):
            tk = tmpbig.tile([128, L], f32, tag="tmp")
            nc.vector.tensor_scalar(
                tk, iota_f, scalar1=idxf[:, k : k + 1], scalar2=w13[:, k : k + 1],
                op0=mybir2.AluOpType.is_equal, op1=mybir2.AluOpType.mult,
            )
            nc.vector.tensor_add(s_t, s_t, tk)

        sT = to_chunks(s_t, "s")
        Sr, Si = fwd(sT, "sp")

        # pointwise: Y = Vhat * conj(Shat)
        u1 = tmpbig.tile([128, L], f32, tag="tmp")
        u2 = tmpbig.tile([128, L], f32, tag="tmp")
        nc.vector.tensor_mul(u1, Vr, Sr)
        nc.vector.tensor_mul(u2, Vi, Si)
        YR = rowbig.tile([128, L], f32, tag="row")
        nc.vector.tensor_add(YR, u1, u2)
        u3 = tmpbig.tile([128, L], f32, tag="tmp")
        u4 = tmpbig.tile([128, L], f32, tag="tmp")
        nc.vector.tensor_mul(u3, Vi, Sr)
        nc.vector.tensor_mul(u4, Vr, Si)
        YI = rowbig.tile([128, L], f32, tag="row")
        nc.vector.tensor_sub(YI, u3, u4)

        YRT = to_chunks(YR, "yr")
        YIT = to_chunks(YI, "yi")
        agg = inv(YRT, YIT, "agg")

        # ---- transpose agg -> [tau, r], replicate 8 heads, DMA out ----
        for tt in range(8):
            tp = psum_t.tile([128, 128], f32, tag="tr")
            nc.tensor.transpose(tp, agg[:, tt * 128 : (tt + 1) * 128], ident)
            at = xtpool.tile([128, 128], f32, tag="at")
            nc.any.tensor_copy(at, tp)
            for b in range(BLOC):
                rep = repp.tile([128, H, DH], f32, tag="rep")
                col = at[:, 64 * b : 64 * b + 64]
                src = bass.AP(
                    tensor=col.tensor, offset=col.offset,
                    ap=[list(col.ap[0]), [0, H], list(col.ap[1])],
                )
                nc.vector.tensor_copy(rep, src)
                nc.sync.dma_start(
                    out=outd[b, tt * 128 : (tt + 1) * 128, :].rearrange(
                        "t (h d) -> t h d", h=H
                    ),
                    in_=rep,
                )

    nc.compile()
    return nc


_cache = threading.Lock(), {}


def _get_nc():
    lock, store = _cache
    with lock:
        if "nc" not in store:
            store["nc"] = _build_nc()
        return store["nc"]


def _make_consts():
    j = np.arange(L, dtype=np.float64)
    ang = 2.0 * np.pi * np.outer(j, j) / L
    Cmat = np.cos(ang).astype(np.float32)
    Smat = (-np.sin(ang)).astype(np.float32)
    return Cmat, Smat


def kernel(Q, K, V, Wq, bq):
    from concourse.bass_utils import run_bass_kernel_spmd

    Q = np.ascontiguousarray(Q, np.float32)
    K = np.ascontiguousarray(K, np.float32)
    V = np.ascontiguousarray(V, np.float32)
    Wq = np.ascontiguousarray(Wq, np.float32)
    bq = np.ascontiguousarray(bq, np.float32)

    nc = _get_nc()
    Cmat, Smat = _make_consts()
    Wql = (Wq / L).astype(np.float32)
    bql = (bq / L).astype(np.float32)

    in_maps = []
    for c in range(NCORES):
        sl = slice(BLOC * c, BLOC * (c + 1))
        in_maps.append(
            {
                "Q": Q[sl], "K": K[sl], "V": V[sl],
                "Wql": Wql, "bql": bql, "Wqu": Wq, "bqu": bq,
                "Cm": Cmat, "Sm": Smat,
            }
        )
    res = run_bass_kernel_spmd(nc, in_maps, list(range(NCORES)))
    return np.concatenate([res.results[i]["out"] for i in range(NCORES)], axis=0)
